# revision 1
# baseline (speedup 1.0000x reference)
"""GNN message-passing kernel for 8 Trainium2 NeuronCores.

Math (per reference):
  h   = relu(ef @ W1 + b1)                      [E, H]
  K   = (h @ W2 + b2).reshape(E, G, L)          per-edge [G, L] kernels
  t   = einsum('bnl,ne->bel', x, inc)           gather nodes->edges
  y   = einsum('egl,bel->beg', K, t)            per-edge matvec
  out = relu(einsum('ne,beg->bng', inc, y) + b_gc).reshape(B, N*G)

Distribution: shard the edge dim E across the 8 cores (2000 edges each).
Every stage (MLP, gather, matvec, scatter-partial) is edge-local; the
scatter partials [B, N, G] are summed on the host (the all-reduce), then
bias + relu applied.

Per-core dataflow (single NEFF, two phases):
  phase 1, per 128-edge chunk: mlp2 -> kT[l,e,g] (bf16), gather
    (xT[n,(b,l)] bf16 x inc[n,e] bf16, K=n accumulated in PSUM) ->
    tT[l,e,b] (bf16), per-edge matmul K=l -> Y psum [g,b], drain to
    Ycp[g,b,e], xbar-DMA-transpose per b -> Yfin[e,(b,g)] bf16, DMA to
    a DRAM staging buffer Y[EL, B*G].
  phase 2: scatter = incT[e,n] x Y[e,(b,g)] with PSUM accumulation over
    all 16 e-chunks (K=2000), 4 node-chunks x 8 (b,g)-chunks, DMA the
    [125, 8, 64] psum tiles straight to out[b,n,g].
"""

import numpy as np
import ml_dtypes

import concourse.bass as bass
from concourse import bacc
import concourse.mybir as mybir
import concourse.tile as tile
from concourse.bass_utils import run_bass_kernel_spmd
from concourse.masks import make_identity

B, N, E, L, G, F, H = 64, 500, 16000, 64, 64, 8, 128
NCORES = 8
ELR = E // NCORES       # 2000 real edges per core
EL = 2048               # padded to a multiple of EC; pad edges have zero
                        # incidence columns so they contribute nothing
EC = 128                # edge chunk
NCH = EL // EC          # 16 chunks
NP = 125                # nodes per n-chunk (500 = 4*125)
NQ = 4                  # n-chunks
BG = B * G              # 4096
F32 = mybir.dt.float32
BF16 = mybir.dt.bfloat16
RELU = mybir.ActivationFunctionType.Relu
IDENT = mybir.ActivationFunctionType.Identity

_CACHE = {}
last_results = None     # BassKernelResults of the most recent run (for test.py)


def _build():
    nc = bacc.Bacc("TRN2", target_bir_lowering=False)
    xT_d = nc.declare_dram_parameter("xT", [N, B * L], BF16, isOutput=False)
    inc_d = nc.declare_dram_parameter("inc", [N, EL], BF16, isOutput=False)
    incT_d = nc.declare_dram_parameter("incT", [EL, N], BF16, isOutput=False)
    efT_d = nc.declare_dram_parameter("efT", [F, EL], BF16, isOutput=False)
    W1_d = nc.declare_dram_parameter("W1", [F, H], BF16, isOutput=False)
    b1_d = nc.declare_dram_parameter("b1", [H, 1], F32, isOutput=False)
    W2_d = nc.declare_dram_parameter("W2", [H, G * L], BF16, isOutput=False)
    b2T_d = nc.declare_dram_parameter("b2T", [H, G * L // H], F32, isOutput=False)
    out_d = nc.declare_dram_parameter("out", [B, N, G], F32, isOutput=True)
    y_d = nc.dram_tensor("Ystage", [EL, BG], BF16)

    with tile.TileContext(nc) as tc, tc.tile_pool(name="const", bufs=1) as cpool:
        with tc.tile_pool(name="h_ps", bufs=2, space="PSUM") as hps:
            # ---- persistent tiles ----
            xT_sb = cpool.tile([NP, NQ, B * L], BF16)       # 32KB/part
            nc.sync.dma_start(
                out=xT_sb[:, :, :],
                in_=xT_d[:, :].rearrange("(q n) c -> n q c", q=NQ),
            )
            W1_sb = cpool.tile([F, H], BF16)
            nc.sync.dma_start(out=W1_sb[:, :], in_=W1_d[:, :])
            b1_sb = cpool.tile([H, 1], F32)
            nc.sync.dma_start(out=b1_sb[:, :], in_=b1_d[:, :])
            W2_sb = cpool.tile([H, G * L], BF16)            # 8KB/part
            nc.sync.dma_start(out=W2_sb[:, :], in_=W2_d[:, :])
            b2T_sb = cpool.tile([H, G * L // H], F32)
            nc.sync.dma_start(out=b2T_sb[:, :], in_=b2T_d[:, :])
            efT_sb = cpool.tile([F, EL], BF16)
            nc.sync.dma_start(out=efT_sb[:, :], in_=efT_d[:, :])
            hT_sb = cpool.tile([H, EL], BF16)               # 4KB/part

            # ---- mlp1: hT = relu(W1.T @ efT + b1), all edges upfront ----
            for c in range(4):
                ph = hps.tile([H, 512], F32)
                nc.tensor.matmul(
                    ph[:, :], lhsT=W1_sb[:, :],
                    rhs=efT_sb[:, c * 512:(c + 1) * 512],
                    start=True, stop=True,
                )
                nc.scalar.activation(
                    hT_sb[:, c * 512:(c + 1) * 512], ph[:, :], RELU,
                    bias=b1_sb[:, 0:1],
                )

        # ---- phase 1 ----
        with (
            tc.tile_pool(name="stream", bufs=2) as spool,
            tc.tile_pool(name="kt", bufs=2) as ktpool,
            tc.tile_pool(name="tt", bufs=2) as ttpool,
            tc.tile_pool(name="ycp", bufs=2) as ycppool,
            tc.tile_pool(name="yfin", bufs=2) as yfpool,
            tc.tile_pool(name="tid", bufs=1) as idpool,
            tc.tile_pool(name="mlp2_ps", bufs=2, space="PSUM") as mps,
            tc.tile_pool(name="gat_ps", bufs=2, space="PSUM") as gps,
            tc.tile_pool(name="mv_ps", bufs=2, space="PSUM") as vps,
            tc.tile_pool(name="tr_ps", bufs=2, space="PSUM") as tps,
        ):
            ident = idpool.tile([L, L], BF16)
            make_identity(nc, ident[:, :])
            for ch in range(NCH):
                e0 = ch * EC
                # mlp2 -> kT[l, g, e] bf16 (+b2); contiguous [64,EC] drains
                kT = ktpool.tile([L, G, EC], BF16, tag="kt")
                for mc in range(32):
                    pm = mps.tile([H, EC], F32, tag="m2")
                    nc.tensor.matmul(
                        pm[:, :], lhsT=W2_sb[:, mc * H:(mc + 1) * H],
                        rhs=hT_sb[:, e0:e0 + EC], start=True, stop=True,
                    )
                    for par in (0, 1):
                        src = pm[par * 64:(par + 1) * 64, :]
                        dst = kT[:, 2 * mc + par, :]
                        bias = b2T_sb[par * 64:(par + 1) * 64, mc:mc + 1]
                        if mc % 2 == 0:
                            nc.scalar.activation(dst, src, IDENT, bias=bias)
                        else:
                            nc.vector.tensor_scalar_add(dst, src, bias)

                # gather -> tT[l, b, e] bf16; contiguous [64,EC] drains
                inc_t = spool.tile([NP, NQ, EC], BF16, tag="inc")
                nc.sync.dma_start(
                    out=inc_t[:, :, :],
                    in_=inc_d[:, e0:e0 + EC].rearrange("(q n) e -> n q e", q=NQ),
                )
                tT = ttpool.tile([L, B, EC], BF16, tag="tt")
                for bp in range(B // 2):
                    pg = gps.tile([2 * L, EC], F32, tag="g")
                    for q in range(NQ):
                        nc.tensor.matmul(
                            pg[:, :],
                            lhsT=xT_sb[:, q, bp * 128:(bp + 1) * 128],
                            rhs=inc_t[:, q, :],
                            start=(q == 0), stop=(q == NQ - 1),
                        )
                    for par in (0, 1):
                        src = pg[par * 64:(par + 1) * 64, :]
                        dst = tT[:, 2 * bp + par, :]
                        if bp % 2 == 0:
                            nc.scalar.copy(dst, src)
                        else:
                            nc.vector.tensor_copy(dst, src)

                # per-edge matvec: psum [g, 8e, b] -> straight-copy drains
                # into ycp[g, j, 8e, b] (raw psum order, e-group-major)
                ycp = ycppool.tile([G, EC // 8, 8, B], BF16, tag="ycp")
                for j in range(EC // 8):
                    pv = vps.tile([G, 8, B], F32, tag="mv")
                    for k in range(8):
                        er = j * 8 + k
                        nc.tensor.matmul(
                            pv[:, k, :], lhsT=kT[:, :, er], rhs=tT[:, :, er],
                            start=True, stop=True,
                        )
                    if j % 2 == 0:
                        nc.scalar.copy(ycp[:, j, :, :], pv[:, :, :])
                    else:
                        nc.vector.tensor_copy(ycp[:, j, :, :], pv[:, :, :])

                # PE transpose per b: [g, e] -> [e, g]; drain full banks
                yfin = yfpool.tile([EC, B, G], BF16, tag="yf")
                for b8 in range(B // 8):
                    pt = tps.tile([EC, 8, G], BF16, tag="tr")
                    for i in range(8):
                        b = b8 * 8 + i
                        nc.tensor.transpose(
                            pt[:, i, :], ycp[:, :, :, b], ident[:, :],
                        )
                    if b8 % 2 == 0:
                        nc.vector.tensor_copy(
                            yfin[:, b8 * 8:(b8 + 1) * 8, :], pt[:, :, :])
                    else:
                        nc.scalar.copy(
                            yfin[:, b8 * 8:(b8 + 1) * 8, :], pt[:, :, :])
                nc.sync.dma_start(
                    out=y_d[e0:e0 + EC, :],
                    in_=yfin[:, :, :],
                )

        # ---- phase 2: scatter with PSUM accumulation over all edges ----
        with (
            tc.tile_pool(name="p2c", bufs=1) as p2c,
            tc.tile_pool(name="p2rhs", bufs=3) as p2r,
            tc.tile_pool(name="acc_ps", bufs=8, space="PSUM") as aps,
        ):
            incT_sb = p2c.tile([EC, NCH, N], BF16)          # 16KB/part
            nc.sync.dma_start(
                out=incT_sb[:, :, :],
                in_=incT_d[:, :].rearrange("(c e) n -> e c n", c=NCH),
            )
            for nj in range(BG // 512):
                paccs = [aps.tile([NP, 8, G], F32, tag="acc", name=f"acc{nj}_{m}")
                         for m in range(NQ)]
                for ec in range(NCH):
                    rt = p2r.tile([EC, 512], BF16, tag="rhs")
                    nc.sync.dma_start(
                        out=rt[:, :],
                        in_=y_d[ec * EC:(ec + 1) * EC, nj * 512:(nj + 1) * 512],
                    )
                    for m in range(NQ):
                        nc.tensor.matmul(
                            paccs[m][:, :, :],
                            lhsT=incT_sb[:, ec, m * NP:(m + 1) * NP],
                            rhs=rt[:, :],
                            start=(ec == 0), stop=(ec == NCH - 1),
                        )
                for m in range(NQ):
                    ot = p2r.tile([NP, 8, G], F32, tag="ostage", name=f"ost{nj}_{m}")
                    if m % 2 == 0:
                        nc.vector.tensor_copy(ot[:, :, :], paccs[m][:, :, :])
                    else:
                        nc.scalar.copy(ot[:, :, :], paccs[m][:, :, :])
                    nc.sync.dma_start(
                        out=out_d[nj * 8:(nj + 1) * 8,
                                  m * NP:(m + 1) * NP, :].transpose([1, 0, 2]),
                        in_=ot[:, :, :],
                    )
    nc.compile()
    return nc


def kernel(x, incidence, ef, W1, b1, W2, b2, b_gc):
    global last_results
    x = np.asarray(x, dtype=np.float32)
    incidence = np.asarray(incidence, dtype=np.float32)
    ef = np.asarray(ef, dtype=np.float32)
    W1 = np.asarray(W1, dtype=np.float32)
    b1 = np.asarray(b1, dtype=np.float32)
    W2 = np.asarray(W2, dtype=np.float32)
    b2 = np.asarray(b2, dtype=np.float32)
    b_gc = np.asarray(b_gc, dtype=np.float32)

    if "nc" not in _CACHE:
        _CACHE["nc"] = _build()
    nc = _CACHE["nc"]

    bf = ml_dtypes.bfloat16
    xT = np.ascontiguousarray(
        x.transpose(1, 0, 2).reshape(N, B * L)).astype(bf)
    inc_bf = incidence.astype(bf)
    incT_bf = np.ascontiguousarray(incidence.T).astype(bf)
    efT = np.ascontiguousarray(ef.T).astype(bf)
    b1c = np.ascontiguousarray(b1.reshape(H, 1))
    W2_bf = W2.astype(bf)
    b2T = np.ascontiguousarray(b2.reshape(G * L // H, H).T)

    pad = EL - ELR
    in_maps = []
    for c in range(NCORES):
        es = slice(c * ELR, (c + 1) * ELR)
        in_maps.append({
            "xT": xT,
            "inc": np.ascontiguousarray(
                np.pad(inc_bf[:, es], ((0, 0), (0, pad)))),
            "incT": np.ascontiguousarray(
                np.pad(incT_bf[es, :], ((0, pad), (0, 0)))),
            "efT": np.ascontiguousarray(
                np.pad(efT[:, es], ((0, 0), (0, pad)))),
            "W1": W1.astype(bf), "b1": b1c, "W2": W2_bf, "b2T": b2T,
        })

    import os
    trace = bool(int(os.environ.get("KERNEL_TRACE", "0")))
    last_results = run_bass_kernel_spmd(
        nc, in_maps, list(range(NCORES)), trace=trace)
    partial = np.zeros((B, N, G), np.float32)
    for r in last_results.results:
        partial += r["out"]
    out = np.maximum(partial + b_gc.reshape(1, 1, G), 0.0)
    return out.reshape(B, N * G).astype(np.float32)



# revision 2
# speedup vs baseline: 1.4490x; 1.4490x over previous
"""GNN message-passing kernel for 8 Trainium2 NeuronCores — v2.

Math (per reference):
  h   = relu(ef @ W1 + b1)                      [E, H]
  K   = (h @ W2 + b2).reshape(E, G, L)          per-edge [G, L] kernels
  t   = einsum('bnl,ne->bel', x, inc)           gather nodes->edges
  y   = einsum('egl,bel->beg', K, t)            per-edge matvec
  out = relu(einsum('ne,beg->bng', inc, y) + b_gc).reshape(B, N*G)

Distribution: shard E across 8 cores (2000 edges each, padded to 2048);
scatter partials summed on host.

v2 layout (all per core, edges processed in 4 superchunks of 512):
  e_local = sc*512 + p*256 + bh*128 + pair   (p = partition-half bit)
  - mlp2/gather emit N=512 matmuls into [128,512] psum; 4 [64,256]
    drains each into kT2/tT2 [(p,l), g|b, pair] so the matvec can run
    2 edges concurrently in PE quadrants (0,0)+(64,64).
  - matvec psum pq [128=(p,g), 8 pair, 64 b]; full [128,512] drains.
  - 128x128 PE transposes per (bh, b): ycp[(p,g), pair, b] ->
    yfin[pair, (p,b,g)], DMA'd contiguously to y_d[blk].
  - scatter in 2 halves (blocks 0-3 / 4-7) so half 1 overlaps phase 1;
    lhsT = incPE[pair, blk, p, n]; rhs = y_d rows; out[2, N, B*G] f32,
    host sums halves + relu + bias.
"""

import numpy as np
import ml_dtypes

import concourse.bass as bass
from concourse import bacc
import concourse.mybir as mybir
import concourse.tile as tile
from concourse.bass_utils import run_bass_kernel_spmd
from concourse.masks import make_identity

B, N, E, L, G, F, H = 64, 500, 16000, 64, 64, 8, 128
NCORES = 8
ELR = E // NCORES       # 2000 real edges per core
EL = 2048               # padded; pad edges have zero incidence
SC = 512                # edges per superchunk
NSC = EL // SC          # 4
PR = 256                # pairs per superchunk
BG = B * G              # 4096
F32 = mybir.dt.float32
BF16 = mybir.dt.bfloat16
RELU = mybir.ActivationFunctionType.Relu
IDENT = mybir.ActivationFunctionType.Identity

_CACHE = {}
last_results = None     # BassKernelResults of the most recent run (for test.py)


def _build():
    nc = bacc.Bacc("TRN2", target_bir_lowering=False)
    xT_d = nc.declare_dram_parameter("xT", [N, B * L], BF16, isOutput=False)
    inc_d = nc.declare_dram_parameter("inc", [N, EL], BF16, isOutput=False)
    incPE_d = nc.declare_dram_parameter("incPE", [128, 8, 2, N], BF16, isOutput=False)
    efT_d = nc.declare_dram_parameter("efT", [F, EL], BF16, isOutput=False)
    W1_d = nc.declare_dram_parameter("W1", [F, H], BF16, isOutput=False)
    b1_d = nc.declare_dram_parameter("b1", [H, 1], F32, isOutput=False)
    W2_d = nc.declare_dram_parameter("W2", [H, G * L], BF16, isOutput=False)
    b2T_d = nc.declare_dram_parameter("b2T", [H, G * L // H], F32, isOutput=False)
    out_d = nc.declare_dram_parameter("out", [2, N, BG], F32, isOutput=True)
    y_d = nc.dram_tensor("Ystage", [8, 128, 2 * BG], BF16)

    with (
        tile.TileContext(nc) as tc,
        tc.tile_pool(name="const", bufs=1) as cpool,
        tc.tile_pool(name="inct", bufs=2) as ipool,
        tc.tile_pool(name="kt", bufs=1) as ktpool,
        tc.tile_pool(name="tt", bufs=1) as ttpool,
        tc.tile_pool(name="ycp", bufs=2) as ycppool,
        tc.tile_pool(name="yfin", bufs=1) as yfpool,
        tc.tile_pool(name="rt", bufs=10) as rtpool,
        tc.tile_pool(name="ot", bufs=2) as otpool,
        tc.tile_pool(name="mm_ps", bufs=3, space="PSUM") as mps,
        tc.tile_pool(name="pq_ps", bufs=2, space="PSUM") as qps,
        tc.tile_pool(name="pt_ps", bufs=2, space="PSUM") as tps,
        tc.tile_pool(name="sc_ps", bufs=1, space="PSUM") as sps,
    ):
        # ---- persistent tiles ----
        xT_sb = cpool.tile([125, 4, B * L], BF16)       # 32KB/part
        nc.sync.dma_start(
            out=xT_sb[:, :, :],
            in_=xT_d[:, :].rearrange("(q n) c -> n q c", q=4),
        )
        W1_sb = cpool.tile([F, H], BF16)
        nc.sync.dma_start(out=W1_sb[:, :], in_=W1_d[:, :])
        b1_sb = cpool.tile([H, 1], F32)
        nc.sync.dma_start(out=b1_sb[:, :], in_=b1_d[:, :])
        W2_sb = cpool.tile([H, G * L], BF16)            # 8KB/part
        nc.sync.dma_start(out=W2_sb[:, :], in_=W2_d[:, :])
        b2T_sb = cpool.tile([H, G * L // H], F32)
        nc.sync.dma_start(out=b2T_sb[:, :], in_=b2T_d[:, :])
        efT_sb = cpool.tile([F, EL], BF16)
        nc.sync.dma_start(out=efT_sb[:, :], in_=efT_d[:, :])
        incPE_sb = cpool.tile([128, 8, 2, N], BF16)     # 16KB/part
        nc.sync.dma_start(out=incPE_sb[:, :, :, :], in_=incPE_d[:, :, :, :])
        hT_sb = cpool.tile([H, EL], BF16)               # 4KB/part
        ident = cpool.tile([128, 128], BF16)
        make_identity(nc, ident[:, :])

        # ---- mlp1: hT = relu(W1.T @ efT + b1) ----
        for c in range(4):
            ph = mps.tile([H, 512], F32, tag="big", name=f"ph{c}")
            nc.tensor.matmul(
                ph[:, :], lhsT=W1_sb[:, :],
                rhs=efT_sb[:, c * 512:(c + 1) * 512],
                start=True, stop=True,
            )
            nc.scalar.activation(
                hT_sb[:, c * 512:(c + 1) * 512], ph[:, :], RELU,
                bias=b1_sb[:, 0:1],
            )

        def scatter_half(h):
            # blocks 4h..4h+3; psum-chain over (blk, p); out rows m*125.
            for nj in range(8):
                rts = []
                for blk in range(4 * h, 4 * h + 4):
                    for p in range(2):
                        rt = rtpool.tile([128, 512], BF16, tag="rt",
                                         name=f"rt{h}_{nj}_{blk}_{p}")
                        nc.sync.dma_start(
                            out=rt[:, :],
                            in_=y_d[blk, :, p * BG + nj * 512:
                                    p * BG + (nj + 1) * 512],
                        )
                        rts.append(rt)
                for m in range(4):
                    ps = sps.tile([125, 512], F32, tag="ps", name=f"ps{h}_{nj}_{m}")
                    k = 0
                    for bi, blk in enumerate(range(4 * h, 4 * h + 4)):
                        for p in range(2):
                            nc.tensor.matmul(
                                ps[:, :],
                                lhsT=incPE_sb[:, blk, p, m * 125:(m + 1) * 125],
                                rhs=rts[bi * 2 + p][:, :],
                                start=(k == 0), stop=(k == 7),
                            )
                            k += 1
                    ot = otpool.tile([125, 512], F32, tag="ot",
                                     name=f"ot{h}_{nj}_{m}")
                    if m % 2 == 0:
                        nc.vector.tensor_copy(ot[:, :], ps[:, :])
                    else:
                        nc.scalar.copy(ot[:, :], ps[:, :])
                    nc.sync.dma_start(
                        out=out_d[h, m * 125:(m + 1) * 125,
                                  nj * 512:(nj + 1) * 512],
                        in_=ot[:, :],
                    )

        for sc in range(NSC):
            e0 = sc * SC
            # ---- mlp2 -> kT2[(p,l), g, pair] (+ b2 bias) ----
            kT2 = ktpool.tile([128, G, PR], BF16, tag="kt", name=f"kt{sc}")
            for mc in range(32):
                pm = mps.tile([H, 512], F32, tag="big", name=f"pm{sc}_{mc}")
                nc.tensor.matmul(
                    pm[:, :], lhsT=W2_sb[:, mc * H:(mc + 1) * H],
                    rhs=hT_sb[:, e0:e0 + SC], start=True, stop=True,
                )
                for par in (0, 1):
                    bias = b2T_sb[par * 64:(par + 1) * 64, mc:mc + 1]
                    for p in (0, 1):
                        src = pm[par * 64:(par + 1) * 64, p * 256:(p + 1) * 256]
                        dst = kT2[p * 64:(p + 1) * 64, 2 * mc + par, :]
                        if p == 0:
                            nc.scalar.activation(dst, src, IDENT, bias=bias)
                        else:
                            nc.vector.tensor_scalar_add(dst, src, bias)

            # ---- gather -> tT2[(p,l), b, pair] ----
            inc_t = ipool.tile([125, 4, SC], BF16, tag="inc", name=f"inc{sc}")
            nc.sync.dma_start(
                out=inc_t[:, :, :],
                in_=inc_d[:, e0:e0 + SC].rearrange("(q n) e -> n q e", q=4),
            )
            tT2 = ttpool.tile([128, B, PR], BF16, tag="tt", name=f"tt{sc}")
            for bp in range(32):
                pg = mps.tile([128, 512], F32, tag="big", name=f"pg{sc}_{bp}")
                for q in range(4):
                    nc.tensor.matmul(
                        pg[:, :],
                        lhsT=xT_sb[:, q, bp * 128:(bp + 1) * 128],
                        rhs=inc_t[:, q, :],
                        start=(q == 0), stop=(q == 3),
                    )
                for b01 in (0, 1):
                    for p in (0, 1):
                        src = pg[b01 * 64:(b01 + 1) * 64, p * 256:(p + 1) * 256]
                        dst = tT2[p * 64:(p + 1) * 64, 2 * bp + b01, :]
                        if p == 0:
                            nc.scalar.copy(dst, src)
                        else:
                            nc.vector.tensor_copy(dst, src)

            # ---- matvec: 2 edges (p halves) concurrent in PE quadrants ----
            # ---- + 128x128 transposes -> yfin[pair, (p,b,g)] -> y_d ----
            for bh in range(2):
                blk = sc * 2 + bh
                ycp = ycppool.tile([128, 128, B], BF16, tag="ycp",
                                   name=f"ycp{sc}_{bh}")
                for pr8 in range(16):
                    pq = qps.tile([128, 8, B], F32, tag="pq",
                                  name=f"pq{sc}_{bh}_{pr8}")
                    for k in range(8):
                        pr = bh * 128 + pr8 * 8 + k
                        for p in (0, 1):
                            nc.tensor.matmul(
                                pq[p * 64:(p + 1) * 64, k, :],
                                lhsT=kT2[p * 64:(p + 1) * 64, :, pr],
                                rhs=tT2[p * 64:(p + 1) * 64, :, pr],
                                start=True, stop=True,
                            )
                    dst = ycp[:, pr8 * 8:(pr8 + 1) * 8, :]
                    if pr8 % 2 == 0:
                        nc.scalar.copy(dst, pq[:, :, :])
                    else:
                        nc.vector.tensor_copy(dst, pq[:, :, :])

                yfin = yfpool.tile([128, 2, B, G], BF16, tag="yf",
                                   name=f"yf{sc}_{bh}")
                for b4 in range(16):
                    pt = tps.tile([128, 4, 2, G], BF16, tag="pt",
                                  name=f"pt{sc}_{bh}_{b4}")
                    for i in range(4):
                        b = b4 * 4 + i
                        nc.tensor.transpose(
                            pt[:, i, :, :], ycp[:, :, b], ident[:, :],
                        )
                    dst = yfin[:, :, b4 * 4:(b4 + 1) * 4, :].transpose([0, 2, 1, 3])
                    if b4 % 2 == 0:
                        nc.vector.tensor_copy(dst, pt[:, :, :, :])
                    else:
                        nc.scalar.copy(dst, pt[:, :, :, :])
                nc.sync.dma_start(
                    out=y_d[blk, :, :],
                    in_=yfin[:, :, :, :],
                )

            if sc == 1:
                scatter_half(0)
        scatter_half(1)

    nc.compile()
    return nc


def kernel(x, incidence, ef, W1, b1, W2, b2, b_gc):
    global last_results
    x = np.asarray(x, dtype=np.float32)
    incidence = np.asarray(incidence, dtype=np.float32)
    ef = np.asarray(ef, dtype=np.float32)
    W1 = np.asarray(W1, dtype=np.float32)
    b1 = np.asarray(b1, dtype=np.float32)
    W2 = np.asarray(W2, dtype=np.float32)
    b2 = np.asarray(b2, dtype=np.float32)
    b_gc = np.asarray(b_gc, dtype=np.float32)

    if "nc" not in _CACHE:
        _CACHE["nc"] = _build()
    nc = _CACHE["nc"]

    bf = ml_dtypes.bfloat16
    xT = np.ascontiguousarray(
        x.transpose(1, 0, 2).reshape(N, B * L)).astype(bf)
    inc_bf = incidence.astype(bf)
    b1c = np.ascontiguousarray(b1.reshape(H, 1))
    W2_bf = W2.astype(bf)
    b2T = np.ascontiguousarray(b2.reshape(G * L // H, H).T)
    efT_full = np.ascontiguousarray(ef.T).astype(bf)

    pad = EL - ELR
    in_maps = []
    for c in range(NCORES):
        es = slice(c * ELR, (c + 1) * ELR)
        inc_c = np.pad(inc_bf[:, es], ((0, 0), (0, pad)))        # [N, EL]
        # incPE[pair, blk, p, n] = inc_c[n, sc*512 + p*256 + bh*128 + pair]
        # with blk = sc*2 + bh
        e_idx = (np.arange(4)[:, None, None] * 512          # sc
                 + np.arange(2)[None, :, None] * 256        # p
                 + np.arange(256)[None, None, :])           # bh*128 + pair
        e_idx = e_idx.reshape(4, 2, 2, 128)                 # [sc, p, bh, pair]
        incPE = inc_c.T[e_idx]                              # [sc, p, bh, pair, N]
        incPE = np.ascontiguousarray(
            incPE.transpose(3, 0, 2, 1, 4).reshape(128, 8, 2, N))
        in_maps.append({
            "xT": xT,
            "inc": np.ascontiguousarray(inc_c),
            "incPE": incPE,
            "efT": np.ascontiguousarray(
                np.pad(efT_full[:, es], ((0, 0), (0, pad)))),
            "W1": W1.astype(bf), "b1": b1c, "W2": W2_bf, "b2T": b2T,
        })

    import os
    trace = bool(int(os.environ.get("KERNEL_TRACE", "0")))
    last_results = run_bass_kernel_spmd(
        nc, in_maps, list(range(NCORES)), trace=trace)
    partial = np.zeros((N, B, G), np.float32)
    for r in last_results.results:
        o = r["out"]                                        # [2, N, BG]
        partial += (o[0] + o[1]).reshape(N, B, G)
    out = np.maximum(partial.transpose(1, 0, 2)
                     + b_gc.reshape(1, 1, G), 0.0)
    return out.reshape(B, N * G).astype(np.float32)
